# revision 11
# baseline (speedup 1.0000x reference)
"""Causal multi-head self-attention block (B=2, T=2048, C=1024, H=16) on 8
Trainium2 NeuronCores.

Sharding: core c = 4*b + g handles batch b (2-way data parallel) and head
group g (4-way tensor parallel over the 16 heads -> 4 heads/core).
c_attn is column-sharded (each core computes K/Q/V features only for its 4
heads); c_proj is row-sharded (each core contracts its 4 heads' attn output
against the matching w_proj columns and emits a full-width partial output).
The 4 partial outputs per batch are summed on the host (+ b_proj).

Per-core device pipeline (all matmuls bf16 with fp32 PSUM accumulation):
  1. KQ^T = (w_kq x)        -> [feat, T] layout, feat on partitions
  2. V    = (x^T w_v^T)     -> [T, d] natural layout, augmented with a
     ones column so the AV matmul also yields the softmax denominators
  3. per head pair, per 512-wide q chunk, over live (causal) k tiles:
       aff^T[k,q] = K^T.T Q^T   (two heads row-packed in the PE array,
       written into the two banks of one PSUM pair tile)
       E = exp(0.125*aff^T)     (ONE ScalarE activate over both banks,
       cast bf16; diagonal tiles masked with one DVE mul)
       [attn^T unnorm; sums] += V_aug.T E   (M=65, per head)
     finalize (deferred into the next block so the PE never waits):
     copy the two sums rows to SBUF as bf16, broadcast them to the head
     partitions with one bf16 selector matmul, take a full-width
     approximate reciprocal on the DVE, normalize.
  4. out_partial = attn^T.T w_proj_slice -> [T, C] bf16, DMA to HBM.

Scheduling notes (each engine executes its stream strictly in order, so
emission order is scheduling):
  - per q-chunk j: KQ tiles, V tiles, attention block for head pair 0,
    then pair 1. The KQ/V/proj matmuls pad the PE during the
    ScalarE-governed attention steps so the PE HAM clock stays warm.
  - the AV matmul of tile i is emitted DELAY tiles after its aff matmul,
    so the PE never waits on ScalarE exp or on the previous block's
    finalize (the pav bank-reuse dependency).
  - input DMA is split across both HWDGE rings (sync + scalar) in
    consumption order; w_kq is laid out per-feature-tile so the first
    attention block can start ~4us in.
  - output partials are written bf16 (summed fp32 on the host).
"""

import os
import sys

for _p in ("/opt/trn_rl_repo",):
    if os.path.isdir(_p) and _p not in sys.path:
        sys.path.append(_p)

import numpy as np
import ml_dtypes

B, T, C, H, D = 2, 2048, 1024, 16, 64
N_CORES = 8
HPC = H // 4          # heads per core = 4
CPC = HPC * D         # attn feature cols per core = 256
KQF = 2 * CPC         # K+Q features per core = 512
TCH = 512             # q-chunk width
NJ = T // TCH         # 4 q chunks
NTI = T // 128        # 16 t tiles
DELAY = 4             # aff->av emission lag (tiles)

_CACHE = {}


def _build_program():
    from contextlib import ExitStack

    import concourse.bass as bass
    import concourse.mybir as mybir
    import concourse.tile as tile
    from concourse import bacc
    from concourse.bass import ts

    f32 = mybir.dt.float32
    bf16 = mybir.dt.bfloat16
    Exp = mybir.ActivationFunctionType.Exp

    nc = bacc.Bacc("TRN2", target_bir_lowering=False, debug=False,
                   num_devices=1)

    xT_d = nc.dram_tensor("xT", [128, 8, T], bf16, kind="ExternalInput")
    wkq_d = nc.dram_tensor("wkq", [128, 4, 8, 128], bf16,
                           kind="ExternalInput")
    bkq_d = nc.dram_tensor("bkq", [128, 4], f32, kind="ExternalInput")
    wv_d = nc.dram_tensor("wv", [128, 8, CPC], bf16, kind="ExternalInput")
    wp_d = nc.dram_tensor("wp", [128, 2, C], bf16, kind="ExternalInput")
    mask_d = nc.dram_tensor("mask", [128, 128], bf16, kind="ExternalInput")
    out_d = nc.dram_tensor("out", [T, C], bf16, kind="ExternalOutput")

    with tile.TileContext(nc) as tc, ExitStack() as ctx:
        pp = ctx.enter_context(tc.tile_pool(name="persist", bufs=1))
        o_pool = ctx.enter_context(tc.tile_pool(name="outp", bufs=1))
        xT_sb = pp.tile([128, 8, T], bf16)
        wkq_sb = pp.tile([128, 4, 8, 128], bf16)
        bkq_sb = pp.tile([128, 4], f32)
        wv_sb = pp.tile([128, 8, CPC], bf16)
        wp_sb = pp.tile([128, 2, C], bf16)
        mask_sb = pp.tile([128, 128], bf16)
        kq_sb = pp.tile([128, 4, T], bf16)
        v_sb = pp.tile([128, NTI, HPC, D + 1], bf16)
        attn_sb = pp.tile([128, 2, T], bf16)
        # per-head-pair softmax denominator rows at partitions 0 and 32
        # (engine ops need 32-aligned partition bases); sel2 broadcasts
        # them to partitions 0:64 / 64:128 via one K=33 bf16 matmul
        s2_sb = pp.tile([33, 2 * NJ * TCH], bf16)
        sel2_sb = pp.tile([33, 128], bf16)

        # input DMA on both HWDGE rings, in consumption order
        nc.scalar.dma_start(bkq_sb[:], bkq_d[:])
        nc.sync.dma_start(wkq_sb[:, 0], wkq_d[:, 0])
        nc.scalar.dma_start(wkq_sb[:, 2], wkq_d[:, 2])
        for c in range(0, 8, 2):
            nc.sync.dma_start(xT_sb[:, c, 0:TCH], xT_d[:, c, 0:TCH])
            nc.scalar.dma_start(xT_sb[:, c + 1, 0:TCH],
                                xT_d[:, c + 1, 0:TCH])
        nc.scalar.dma_start(wv_sb[:], wv_d[:])
        nc.scalar.dma_start(mask_sb[:], mask_d[:])
        nc.sync.dma_start(wkq_sb[:, 1], wkq_d[:, 1])
        nc.sync.dma_start(wkq_sb[:, 3], wkq_d[:, 3])
        for ch in range(2):
            nc.scalar.dma_start(xT_sb[:, ts(ch, 4), ts(1, TCH)],
                                xT_d[:, ts(ch, 4), ts(1, TCH)])
        nc.sync.dma_start(xT_sb[:, :, ts(2, TCH)], xT_d[:, :, ts(2, TCH)])
        nc.sync.dma_start(xT_sb[:, :, ts(3, TCH)], xT_d[:, :, ts(3, TCH)])
        nc.sync.dma_start(wp_sb[:], wp_d[:])

        for ti in range(NTI):
            nc.any.memset(v_sb[:, ti, :, D:D + 1], 1.0)
        nc.any.memset(s2_sb[:], 0.0)
        nc.any.memset(sel2_sb[:], 0.0)
        nc.any.memset(sel2_sb[0:1, 0:64], 1.0)
        nc.any.memset(sel2_sb[32:33, 64:128], 1.0)

        # One shared PSUM pool: 8 banks = aff pair tiles (2x2) + acc(2)
        # + work(2).
        pa_pool = ctx.enter_context(
            tc.tile_pool(name="pall", bufs=1, space="PSUM"))
        e_pool = ctx.enter_context(tc.tile_pool(name="epool", bufs=1))
        r_pool = ctx.enter_context(tc.tile_pool(name="rpool", bufs=1))

        def emit_kq_tiles(ms, tch):
            # interleave the c-loops of several feature tiles so the PE can
            # advance as each 128-row chunk of x arrives from HBM
            pk = {m: pa_pool.tile([128, TCH], f32, tag="work", bufs=2,
                                  name="pkq") for m in ms}
            for c in range(8):
                for m in ms:
                    nc.tensor.matmul(
                        pk[m][:], wkq_sb[:, m, c, :],
                        xT_sb[:, c, ts(tch, TCH)],
                        start=(c == 0), stop=(c == 7))
            for m in ms:
                nc.vector.tensor_scalar_add(
                    kq_sb[:, m, ts(tch, TCH)], pk[m][:], bkq_sb[:, m:m + 1])

        def emit_v(tis):
            for ti in tis:
                pv = pa_pool.tile([128, CPC], f32, tag="work", bufs=2,
                                  name="pv")
                for c in range(8):
                    nc.tensor.matmul(
                        pv[:], xT_sb[:, c, ts(ti, 128)], wv_sb[:, c, :],
                        start=(c == 0), stop=(c == 7))
                nc.vector.tensor_copy(
                    v_sb[:, ti, :, 0:D],
                    pv[:].rearrange("p (h d) -> p h d", h=HPC))

        def emit_attn_block(g, j, hooks=()):
            """Emit one (head-pair, q-chunk) attention block.

            `hooks` is a list of (i, fn): fn is emitted after emit_aff(i).
            Returns (finalize_A, finalize_B) closures the caller schedules
            inside the NEXT emitted block (A first, B a few tiles later).
            """
            pav0 = pa_pool.tile([128, TCH], f32, tag="acc", bufs=2,
                                name="pav0")
            pav1 = pa_pool.tile([128, TCH], f32, tag="acc", bufs=2,
                                name="pav1")
            n_live = 4 * j + 4
            es = {}

            def emit_aff(i):
                # diagonal tiles only touch queries q >= k: narrow the
                # q-range to [q0:TCH] and mask just its first 128 columns
                q0 = max(0, 128 * i - TCH * j)
                qsl = slice(j * TCH + q0, (j + 1) * TCH)
                ap = pa_pool.tile([128, 2, TCH], f32, tag="aff", bufs=2,
                                  name="ap")
                nc.tensor.matmul(
                    ap[:, 0, q0:], kq_sb[0:64, g, ts(i, 128)],
                    kq_sb[0:64, 2 + g, qsl], start=True, stop=True)
                nc.tensor.matmul(
                    ap[:, 1, q0:], kq_sb[64:128, g, ts(i, 128)],
                    kq_sb[64:128, 2 + g, qsl], start=True, stop=True)
                ep = e_pool.tile([128, 2, TCH], bf16, tag="ep",
                                 bufs=DELAY + 1, name="ep")
                nc.scalar.activation(ep[:, :, q0:], ap[:, :, q0:], Exp,
                                     scale=0.125)
                if q0 > 0 or i == 4 * j:
                    tri = mask_sb[:, 0:128].rearrange(
                        "p (o q) -> p o q", o=1).broadcast_to([128, 2, 128])
                    nc.vector.tensor_mul(
                        ep[:, :, q0:q0 + 128], ep[:, :, q0:q0 + 128], tri)
                es[i] = (ep, q0)

            def emit_av(i):
                ep, q0 = es.pop(i)
                first, last = (i == 0), (i == n_live - 1)
                nc.tensor.matmul(
                    pav0[0:65, q0:], v_sb[:, i, 2 * g + 0, :],
                    ep[:, 0, q0:], start=first, stop=last)
                nc.tensor.matmul(
                    pav1[0:65, q0:], v_sb[:, i, 2 * g + 1, :],
                    ep[:, 1, q0:], start=first, stop=last)

            hooks = sorted(hooks, key=lambda h: h[0])
            for i in range(n_live):
                emit_aff(i)
                for at, fn in hooks:
                    if at == i:
                        fn()
                if i >= DELAY:
                    emit_av(i - DELAY)
            for at, fn in hooks:
                if at >= n_live:
                    fn()
            for i in range(max(0, n_live - DELAY), n_live):
                emit_av(i)

            chunk = ts(g * NJ + j, TCH)

            def finalize_A():
                with nc.allow_low_precision(reason="bf16 softmax sums"):
                    nc.vector.tensor_copy(s2_sb[0:1, chunk], pav0[64:65, :])
                    nc.vector.tensor_copy(s2_sb[32:33, chunk],
                                          pav1[64:65, :])

            def finalize_B():
                pr = pa_pool.tile([128, TCH], f32, tag="work", bufs=2,
                                  name="pr")
                nc.tensor.matmul(pr[:], sel2_sb[:], s2_sb[:, chunk],
                                 start=True, stop=True)
                rbf = r_pool.tile([128, TCH], f32, tag="rb", bufs=2)
                with nc.allow_low_precision(reason="approx softmax recip"):
                    nc.vector.reciprocal_approx_fast(rbf[:], pr[:])
                nc.vector.tensor_mul(
                    attn_sb[0:64, g, ts(j, TCH)], pav0[0:64, :],
                    rbf[0:64, :])
                nc.vector.tensor_mul(
                    attn_sb[64:128, g, ts(j, TCH)], pav1[0:64, :],
                    rbf[64:128, :])

            return finalize_A, finalize_B

        def emit_proj(ti, eng=None):
            for och in range(2):
                po = pa_pool.tile([128, 512], f32, tag="work", bufs=2,
                                  name="po")
                nc.tensor.matmul(
                    po[:], attn_sb[:, 0, ts(ti, 128)],
                    wp_sb[:, 0, ts(och, 512)], start=True, stop=False)
                nc.tensor.matmul(
                    po[:], attn_sb[:, 1, ts(ti, 128)],
                    wp_sb[:, 1, ts(och, 512)], start=False, stop=True)
                ot = o_pool.tile([128, 512], bf16, tag="ot", bufs=4)
                with nc.allow_low_precision(reason="bf16 partial output"):
                    nc.vector.tensor_copy(ot[:], po[:])
                e = eng if eng is not None else nc.sync
                e.dma_start(out_d[ts(ti, 128), ts(och, 512)], ot[:])

        def proj_chunk(j, split_rings=False):
            def h():
                for k, ti in enumerate(range(4 * j, 4 * j + 4)):
                    eng = (nc.scalar if split_rings and k % 2 else None)
                    emit_proj(ti, eng=eng)
            return h

        # schedule: per q-chunk j, the two attention blocks carry (as
        # hooks, so every engine's queue stays primed) the previous
        # block's finalize, the previous chunk's output projection, and
        # the NEXT chunk's KQ/V tiles -- whose DVE bias-adds/copies must
        # land before the next chunk's aff matmuls need them.
        def kq_hook(ms, tch):
            def h():
                emit_kq_tiles(ms, tch)
            return h

        def v_hook(j):
            def h():
                emit_v(range(4 * j, 4 * j + 4))
            return h

        fin = None
        prev_j = None
        emit_kq_tiles([0, 2], 0)
        emit_v(range(0, 4))
        for j in range(NJ):
            hooks = [(0, fin[0]), (3, fin[1])] if fin else []
            hooks.append((5, kq_hook([1, 3], j)))
            fin = emit_attn_block(0, j, hooks=hooks)
            hooks = [(0, fin[0]), (3, fin[1])]
            if prev_j is not None:
                hooks.append((5, proj_chunk(prev_j)))
            if j + 1 < NJ:
                hooks.append((7, kq_hook([0, 2], j + 1)))
                hooks.append((9, v_hook(j + 1)))
            fin = emit_attn_block(1, j, hooks=hooks)
            prev_j = j
        fin[0]()
        fin[1]()
        proj_chunk(prev_j, split_rings=True)()

    nc.compile()
    return nc


def _get_program():
    if "nc" not in _CACHE:
        _CACHE["nc"] = _build_program()
    return _CACHE["nc"]


def _host_mask():
    # tri[i, jj] = 1.0 iff key offset i <= query offset jj within the
    # diagonal 128x128 tile
    i = np.arange(128)[:, None]
    jj = np.arange(128)[None, :]
    return (i <= jj).astype(ml_dtypes.bfloat16)


def _shard_inputs(x, w_attn, b_attn, w_proj, b_proj):
    bf = ml_dtypes.bfloat16
    mask = _host_mask()
    in_maps = []
    for c in range(N_CORES):
        b, g = divmod(c, 4)
        hs = slice(g * CPC, (g + 1) * CPC)
        # xT: (C, T) -> (128, 8, T)
        xT = np.ascontiguousarray(
            x[b].T.reshape(8, 128, T).transpose(1, 0, 2)).astype(bf)
        # K block rows 0:C, Q rows C:2C, V rows 2C:3C of w_attn
        wkq = np.concatenate([w_attn[0 + g * CPC:0 + (g + 1) * CPC],
                              w_attn[C + g * CPC:C + (g + 1) * CPC]], axis=0)
        # (KQF, C) -> (C, KQF) -> [128p, 4m, 8c, 128f]
        wkq = np.ascontiguousarray(
            wkq.T.reshape(8, 128, 4, 128).transpose(1, 2, 0, 3)).astype(bf)
        bkq = np.concatenate([b_attn[0 + g * CPC:0 + (g + 1) * CPC],
                              b_attn[C + g * CPC:C + (g + 1) * CPC]])
        bkq = np.ascontiguousarray(bkq.reshape(4, 128).T).astype(np.float32)
        wv = w_attn[2 * C + g * CPC:2 * C + (g + 1) * CPC]  # (CPC, C)
        wv = np.ascontiguousarray(
            wv.T.reshape(8, 128, CPC).transpose(1, 0, 2)).astype(bf)
        wp = w_proj[:, hs].T  # (CPC, C)
        wp = np.ascontiguousarray(
            wp.reshape(2, 128, C).transpose(1, 0, 2)).astype(bf)
        in_maps.append({"xT": xT, "wkq": wkq, "bkq": bkq, "wv": wv,
                        "wp": wp, "mask": mask})
    return in_maps


def kernel(x, w_attn, b_attn, w_proj, b_proj):
    from concourse.bass_utils import run_bass_kernel_spmd

    nc = _get_program()
    in_maps = _shard_inputs(x, w_attn, b_attn, w_proj, b_proj)
    res = run_bass_kernel_spmd(nc, in_maps, core_ids=list(range(N_CORES)))
    out = np.zeros((B, T, C), dtype=np.float32)
    for c in range(N_CORES):
        b = c // 4
        out[b] += res.results[c]["out"].astype(np.float32)
    # V-bias contribution folded out of the device kernel:
    # (attn + bv)^T @ wp  =  attn^T @ wp  +  (bv @ wp)
    bv_full = b_attn[2 * C:3 * C].astype(np.float64)
    bias_out = bv_full @ w_proj.T.astype(np.float64)
    out += (b_proj.astype(np.float64) + bias_out)[None, None, :].astype(
        np.float32)
    return out
